# revision 11
# baseline (speedup 1.0000x reference)
"""Trainium2 Bass kernel for nn_DynamicSparseAttention (B=2,S=1024,E=1024,H=16,K=64).

Sharding: 8 cores = 2 batches x 4 head-groups (4 heads each).
Per core: QKV projections for its 4 heads (f32 for Q/K since top-k selection is
precision-critical, bf16 for V), per-head scores, exact top-64 per query row via
8 rounds of DVE max8 + match_replace, masked softmax with normalization folded
into the ACT exp bias, DMA-transposed weights -> AV matmul, AllGather of head
outputs within each batch group, then full output projection.
"""

import sys

if "/opt/trn_rl_repo" not in sys.path:
    sys.path.insert(0, "/opt/trn_rl_repo")

import numpy as np
import ml_dtypes


def _install_ntff_hook_module():
    """bass_utils(trace=True) imports antenv.axon_hooks, which this image's
    read-only antenv lacks; provide it via sys.modules (ctypes into
    libaxon_pjrt.so, same recipe as the boot script)."""
    import types, contextlib, ctypes

    if "antenv.axon_hooks" in sys.modules:
        return
    mod = types.ModuleType("antenv.axon_hooks")
    state = {"hook": None}

    def _make_hook(so_path="/opt/axon/libaxon_pjrt.so"):
        lib = ctypes.CDLL(so_path)
        if not hasattr(lib, "axon_start_nrt_profile"):
            return None
        lib.axon_start_nrt_profile.argtypes = [
            ctypes.POINTER(ctypes.c_int64), ctypes.c_size_t]
        lib.axon_start_nrt_profile.restype = ctypes.c_int64
        lib.axon_stop_nrt_profile.argtypes = [ctypes.c_char_p]
        lib.axon_stop_nrt_profile.restype = ctypes.c_int64

        @contextlib.contextmanager
        def _hook(output_dir, device_ids):
            import jax
            jax.devices()
            if device_ids:
                ids = (ctypes.c_int64 * len(device_ids))(*device_ids)
                rc = lib.axon_start_nrt_profile(ids, len(device_ids))
            else:
                rc = lib.axon_start_nrt_profile(None, 0)
            if rc != 0:
                raise RuntimeError(f"axon_start_nrt_profile rc={rc}")
            try:
                yield
            finally:
                n = lib.axon_stop_nrt_profile(str(output_dir).encode())
                print(f"profile: {n} file(s) -> {output_dir}", file=sys.stderr)

        return _hook

    def get_axon_ntff_profile_hook():
        if state["hook"] is None:
            try:
                state["hook"] = _make_hook()
            except OSError:
                state["hook"] = None
        return state["hook"]

    def set_axon_ntff_profile_hook(hook):
        state["hook"] = hook

    mod.get_axon_ntff_profile_hook = get_axon_ntff_profile_hook
    mod.set_axon_ntff_profile_hook = set_axon_ntff_profile_hook
    sys.modules["antenv.axon_hooks"] = mod
    try:
        import antenv
        antenv.axon_hooks = mod
    except ImportError:
        pass


_install_ntff_hook_module()

B, S, E = 2, 1024, 1024
H, HD, TOPK = 16, 64, 64
NCORES = 8
HPC = 4          # heads per core
DL = HPC * HD    # 256 local e dims per core
NT = E // 128    # 8 e-tiles
SCALE = 1.0 / 8.0  # 1/sqrt(hd)

BF16 = ml_dtypes.bfloat16

_CACHE = {}


def _build_nc():
    import concourse.bass as bass
    import concourse.bacc as bacc
    import concourse.tile as tile
    from concourse import mybir

    f32 = mybir.dt.float32
    bf16 = mybir.dt.bfloat16
    AF = mybir.ActivationFunctionType
    OP = mybir.AluOpType

    nc = bacc.Bacc("TRN2", target_bir_lowering=False, debug=False,
                   num_devices=NCORES)

    xT_d = nc.dram_tensor("xT", [E, S], f32, kind="ExternalInput")
    xTb_d = nc.dram_tensor("xTb", [E, S], bf16, kind="ExternalInput")
    wqT_d = nc.dram_tensor("wqT", [E, DL], f32, kind="ExternalInput")
    wkT_d = nc.dram_tensor("wkT", [E, DL], f32, kind="ExternalInput")
    wvT_d = nc.dram_tensor("wvT", [E, DL], bf16, kind="ExternalInput")
    woT_d = nc.dram_tensor("woT", [E, E], bf16, kind="ExternalInput")
    bq_d = nc.dram_tensor("bq", [DL, 1], f32, kind="ExternalInput")
    bk_d = nc.dram_tensor("bk", [DL, 1], f32, kind="ExternalInput")
    bv_d = nc.dram_tensor("bv", [E, 1], bf16, kind="ExternalInput")
    bo_d = nc.dram_tensor("bo", [1, E], f32, kind="ExternalInput")
    y_d = nc.dram_tensor("y", [S, E], f32, kind="ExternalOutput")

    outT_int = [nc.dram_tensor(f"outT_int{i}", [128, S], bf16) for i in range(2)]
    ag_out = [nc.dram_tensor(f"ag_out{i}", [512, S], bf16) for i in range(2)]
    groups = [[0, 1, 2, 3], [4, 5, 6, 7]]

    with tile.TileContext(nc) as tc:
        with tc.tile_pool(name="persist", bufs=1) as pp, \
             tc.tile_pool(name="psum", bufs=1, space="PSUM") as psp:
            qt_sb = [pp.tile([128, S], f32, tag=f"qt{p}", name=f"qtsb{p}")
                     for p in range(2)]
            kt_sb = [pp.tile([128, S], f32, tag=f"kt{p}", name=f"ktsb{p}")
                     for p in range(2)]
            v_sb = pp.tile([128, NT * DL], bf16, tag="v")
            outT_sb = [pp.tile([64, S], bf16, tag=f"ot{h}", name=f"outTsb{h}")
                       for h in range(HPC)]
            ones_sb = pp.tile([1, 128], bf16, tag="ones")
            wo_sb = pp.tile([128, NT * E], bf16, tag="wo")
            bv_sb = pp.tile([128, NT], bf16, tag="bv")
            bo_sb = pp.tile([1, E], f32, tag="bo")
            c_sb = pp.tile([1, E], bf16, tag="c")
            nc.vector.memset(ones_sb[:], 1.0)

            wo_t = woT_d.ap().rearrange("(t p) e -> t p e", p=128)
            bv_t = bv_d.ap().rearrange("(t p) o -> t p o", p=128)
            for t in range(NT):
                nc.sync.dma_start(wo_sb[:, t * E:(t + 1) * E], wo_t[t])
                nc.sync.dma_start(bv_sb[:, t:t + 1], bv_t[t])
            nc.sync.dma_start(bo_sb[:], bo_d.ap())

            # ------------- phase A: projections -------------
            with tc.tile_pool(name="phaseA", bufs=1) as pa:
                xT_sb = pa.tile([128, NT * S], f32, tag="xT")
                xTb_sb = pa.tile([128, NT * S], bf16, tag="xTb")
                wq_sb = pa.tile([128, NT * DL], f32, tag="wq")
                wk_sb = pa.tile([128, NT * DL], f32, tag="wk")
                wv_sb = pa.tile([128, NT * DL], bf16, tag="wv")
                bq_sb = pa.tile([128, 2], f32, tag="bq")
                bk_sb = pa.tile([128, 2], f32, tag="bk")

                xT_t = xT_d.ap().rearrange("(t p) s -> t p s", p=128)
                xTb_t = xTb_d.ap().rearrange("(t p) s -> t p s", p=128)
                wq_t = wqT_d.ap().rearrange("(t p) d -> t p d", p=128)
                wk_t = wkT_d.ap().rearrange("(t p) d -> t p d", p=128)
                wv_t = wvT_d.ap().rearrange("(t p) d -> t p d", p=128)
                for t in range(NT):
                    nc.sync.dma_start(xT_sb[:, t * S:(t + 1) * S], xT_t[t])
                    nc.sync.dma_start(xTb_sb[:, t * S:(t + 1) * S], xTb_t[t])
                    nc.sync.dma_start(wq_sb[:, t * DL:(t + 1) * DL], wq_t[t])
                    nc.sync.dma_start(wk_sb[:, t * DL:(t + 1) * DL], wk_t[t])
                    nc.sync.dma_start(wv_sb[:, t * DL:(t + 1) * DL], wv_t[t])
                bq_t = bq_d.ap().rearrange("(h p) o -> h p o", p=128)
                bk_t = bk_d.ap().rearrange("(h p) o -> h p o", p=128)
                for p in range(2):
                    nc.sync.dma_start(bq_sb[:, p:p + 1], bq_t[p])
                    nc.sync.dma_start(bk_sb[:, p:p + 1], bk_t[p])

                def qk_proj(p):
                    for (w_sb, b_sb, dst) in ((wk_sb, bk_sb, kt_sb),
                                              (wq_sb, bq_sb, qt_sb)):
                        for nb in range(2):
                            ps = psp.tile([128, 512], f32, tag="small",
                                          bufs=2, name=f"pj{p}{nb}")
                            for t in range(NT):
                                nc.tensor.matmul(
                                    ps[:],
                                    w_sb[:, t * DL + p * 128: t * DL + (p + 1) * 128],
                                    xT_sb[:, t * S + nb * 512: t * S + nb * 512 + 512],
                                    start=(t == 0), stop=(t == NT - 1))
                            nc.scalar.activation(
                                dst[p][:, nb * 512:(nb + 1) * 512], ps[:],
                                AF.Identity, bias=b_sb[:, p:p + 1])

                def v_proj():
                    for kt in range(NT):
                        ps = psp.tile([128, DL], f32, tag="small", bufs=2,
                                      name=f"vp{kt}")
                        for t in range(NT):
                            nc.tensor.matmul(
                                ps[:],
                                xTb_sb[:, t * S + kt * 128: t * S + (kt + 1) * 128],
                                wv_sb[:, t * DL:(t + 1) * DL],
                                start=(t == 0), stop=(t == NT - 1))
                        nc.scalar.activation(v_sb[:, kt * DL:(kt + 1) * DL],
                                             ps[:], AF.Copy)

                qk_proj(0)
                v_proj()

                # ------------- phase B: attention per head -------------
                with tc.tile_pool(name="sco", bufs=3) as sco_p, \
                     tc.tile_pool(name="zap", bufs=3) as zap_p, \
                     tc.tile_pool(name="msk", bufs=3) as msk_p, \
                     tc.tile_pool(name="prob", bufs=3) as prob_p, \
                     tc.tile_pool(name="pmw", bufs=3) as pm_p, \
                     tc.tile_pool(name="small", bufs=8) as sm_p, \
                     tc.tile_pool(name="pmT", bufs=2) as pmT_p:
                    for h in range(HPC):
                        pair, sub = h // 2, h % 2
                        r0 = sub * 64
                        if h == 1:
                            qk_proj(1)   # overlap pair-1 proj with head work
                        pmT = pmT_p.tile([128, NT * S], bf16, tag="pmT",
                                         name=f"pmT{h}")
                        for qp in range(NT // 2):
                            qts = (2 * qp, 2 * qp + 1)
                            sps_l, s_l, z_l, t64_l = [], [], [], []
                            for qt in qts:
                                sps = psp.tile([128, S], f32, tag="big",
                                               bufs=3, name=f"sps{h}{qt}")
                                for nb in range(2):
                                    nc.tensor.matmul(
                                        sps[:, nb * 512:(nb + 1) * 512],
                                        qt_sb[pair][r0:r0 + 64, qt * 128:(qt + 1) * 128],
                                        kt_sb[pair][r0:r0 + 64, nb * 512:(nb + 1) * 512],
                                        start=True, stop=True)
                                sps_l.append(sps)
                                s_l.append(sco_p.tile([128, S], f32, tag="s",
                                                      name=f"s{h}{qt}"))
                                z_l.append(zap_p.tile([128, S], f32, tag="z",
                                                      name=f"z{h}{qt}"))
                                t64_l.append(sm_p.tile([128, 64], f32,
                                                       tag=f"t64_{qt % 2}",
                                                       name=f"t64_{h}{qt}"))
                            # interleaved exact top-64 extraction for the two
                            # q-tiles: every needle-load (MATCH_VALUE_LOAD)
                            # hides behind the other tile's big scan, so the
                            # DVE pipe-drain bubbles vanish
                            nc.vector.max(t64_l[0][:, 0:8], sps_l[0][:])
                            nc.vector.max(t64_l[1][:, 0:8], sps_l[1][:])
                            nc.vector.match_replace(z_l[0][:], t64_l[0][:, 0:8],
                                                    sps_l[0][:], -1e30)
                            nc.vector.match_replace(z_l[1][:], t64_l[1][:, 0:8],
                                                    sps_l[1][:], -1e30)
                            nc.scalar.activation(s_l[0][:], sps_l[0][:], AF.Copy)
                            nc.scalar.activation(s_l[1][:], sps_l[1][:], AF.Copy)
                            for r in range(1, 8):
                                nc.vector.max(t64_l[0][:, 8 * r:8 * r + 8],
                                              z_l[0][:])
                                nc.vector.max(t64_l[1][:, 8 * r:8 * r + 8],
                                              z_l[1][:])
                                if r < 7:
                                    nc.vector.match_replace(
                                        z_l[0][:], t64_l[0][:, 8 * r:8 * r + 8],
                                        z_l[0][:], -1e30)
                                    nc.vector.match_replace(
                                        z_l[1][:], t64_l[1][:, 8 * r:8 * r + 8],
                                        z_l[1][:], -1e30)
                            for i, qt in enumerate(qts):
                                t64, s_sb = t64_l[i], s_l[i]
                                theta = t64[:, 63:64]
                                p_sb = prob_p.tile([128, S], bf16, tag="p",
                                                   name=f"p{h}{qt}")
                                nc.scalar.activation(p_sb[:], s_sb[:], AF.Exp,
                                                     scale=SCALE)
                                e64 = sm_p.tile([128, 64], f32, tag="e64",
                                                name=f"e64_{h}{qt}")
                                den = sm_p.tile([128, 1], f32, tag="den",
                                                name=f"den{h}{qt}")
                                nc.scalar.activation(e64[:], t64[:], AF.Exp,
                                                     scale=SCALE,
                                                     accum_out=den[:])
                                rden = sm_p.tile([128, 1], f32, tag="rden",
                                                 name=f"rden{h}{qt}")
                                nc.vector.reciprocal(rden[:], den[:])
                                m_sb = msk_p.tile([128, S], bf16, tag="m",
                                                  name=f"m{h}{qt}")
                                nc.vector.tensor_scalar(m_sb[:], s_sb[:],
                                                        theta, None,
                                                        op0=OP.is_ge)
                                pm_sb = pm_p.tile([128, S], bf16, tag="pm",
                                                  name=f"pm{h}{qt}")
                                nc.vector.scalar_tensor_tensor(
                                    pm_sb[:], p_sb[:], rden[:], m_sb[:],
                                    op0=OP.mult, op1=OP.mult)
                                for kt in range(NT):
                                    nc.sync.dma_start(
                                        pmT[:, kt * S + qt * 128: kt * S + (qt + 1) * 128],
                                        pm_sb[:, kt * 128:(kt + 1) * 128],
                                        transpose=True)
                        avps = psp.tile([64, S], f32, tag="big", bufs=3,
                                        name=f"avps{h}")
                        for nb in range(2):
                            for kt in range(NT):
                                nc.tensor.matmul(
                                    avps[:, nb * 512:(nb + 1) * 512],
                                    v_sb[:, kt * DL + h * 64: kt * DL + (h + 1) * 64],
                                    pmT[:, kt * S + nb * 512: kt * S + nb * 512 + 512],
                                    start=(kt == 0), stop=(kt == NT - 1))
                        nc.scalar.activation(outT_sb[h][:], avps[:], AF.Copy)
                        nc.sync.dma_start(
                            outT_int[h // 2].ap()[(h % 2) * 64:(h % 2) * 64 + 64, :],
                            outT_sb[h][:])
                        if h % 2 == 1:
                            # AllGather this half; the first one overlaps the
                            # remaining heads' compute
                            nc.gpsimd.collective_compute(
                                "AllGather", mybir.AluOpType.bypass,
                                ins=[outT_int[h // 2].ap()],
                                outs=[ag_out[h // 2].ap()],
                                replica_groups=groups)

            # ------------- phase D: output projection -------------
            with tc.tile_pool(name="phaseD", bufs=1) as pd, \
                 tc.tile_pool(name="ysb", bufs=2) as yp:
                ot_sb = pd.tile([128, NT * S], bf16, tag="ot")
                for t in [0, 2, 4, 6, 1, 3, 5, 7]:
                    r, half = t // 2, t % 2
                    nc.sync.dma_start(
                        ot_sb[:, t * S:(t + 1) * S],
                        ag_out[half].ap()[r * 128:(r + 1) * 128, :])

                # c = bv @ Wo.T + bo   (constant row, [1, E])
                cps = psp.tile([1, E], f32, tag="big", bufs=3, name="cps")
                for nb in range(2):
                    for t in range(NT):
                        nc.tensor.matmul(
                            cps[:, nb * 512:(nb + 1) * 512],
                            bv_sb[:, t:t + 1],
                            wo_sb[:, t * E + nb * 512: t * E + nb * 512 + 512],
                            start=(t == 0), stop=(t == NT - 1))
                nc.vector.tensor_tensor(c_sb[:], cps[:], bo_sb[:], op=OP.add)

                for st in range(NT):
                    yps = psp.tile([128, E], f32, tag="big", bufs=3,
                                   name=f"yps{st}")
                    for nb in range(2):
                        for i, t in enumerate([0, 2, 4, 6, 1, 3, 5, 7]):
                            nc.tensor.matmul(
                                yps[:, nb * 512:(nb + 1) * 512],
                                ot_sb[:, t * S + st * 128: t * S + (st + 1) * 128],
                                wo_sb[:, t * E + nb * 512: t * E + nb * 512 + 512],
                                start=(i == 0), stop=False)
                        nc.tensor.matmul(
                            yps[:, nb * 512:(nb + 1) * 512],
                            ones_sb[:],
                            c_sb[:, nb * 512:(nb + 1) * 512],
                            start=False, stop=True)
                    y_sb = yp.tile([128, E], f32, tag="y", name=f"y{st}")
                    nc.scalar.activation(y_sb[:], yps[:], AF.Copy)
                    nc.sync.dma_start(y_d.ap()[st * 128:(st + 1) * 128, :],
                                      y_sb[:])

    nc.compile()
    return nc


def _get_nc():
    if "nc" not in _CACHE:
        _CACHE["nc"] = _build_nc()
    return _CACHE["nc"]


def _in_maps(x, Wq, bq, Wk, bk, Wv, bv, Wo, bo):
    x = np.asarray(x, np.float32)
    Wq = np.asarray(Wq, np.float32)
    Wk = np.asarray(Wk, np.float32)
    Wv = np.asarray(Wv, np.float32)
    Wo = np.asarray(Wo, np.float32)
    bq = np.asarray(bq, np.float32)
    bk = np.asarray(bk, np.float32)
    bv = np.asarray(bv, np.float32)
    bo = np.asarray(bo, np.float32)

    woT = np.ascontiguousarray(Wo.T).astype(BF16)
    bo_r = bo.reshape(1, E)
    bv_r = bv.reshape(E, 1).astype(BF16)
    maps = []
    for c in range(NCORES):
        b = c // 4
        dlo = (c % 4) * DL
        xT = np.ascontiguousarray(x[b].T)
        maps.append({
            "xT": xT,
            "xTb": xT.astype(BF16),
            "wqT": np.ascontiguousarray(Wq[dlo:dlo + DL, :].T),
            "wkT": np.ascontiguousarray(Wk[dlo:dlo + DL, :].T),
            "wvT": np.ascontiguousarray(Wv[dlo:dlo + DL, :].T).astype(BF16),
            "woT": woT,
            "bq": np.ascontiguousarray(bq[dlo:dlo + DL].reshape(DL, 1)),
            "bk": np.ascontiguousarray(bk[dlo:dlo + DL].reshape(DL, 1)),
            "bv": bv_r,
            "bo": bo_r,
        })
    return maps


def run_on_hw(inputs, trace=False):
    """Run the bass kernel; returns (output, BassKernelResults)."""
    from concourse.bass_utils import run_bass_kernel_spmd

    nc = _get_nc()
    maps = _in_maps(**inputs)
    res = run_bass_kernel_spmd(nc, maps, core_ids=list(range(NCORES)),
                               trace=trace)
    y = np.stack([np.asarray(res.results[0]["y"]),
                  np.asarray(res.results[4]["y"])]).astype(np.float32)
    return y, res


def kernel(x, Wq, bq, Wk, bk, Wv, bv, Wo, bo):
    y, _ = run_on_hw(dict(x=x, Wq=Wq, bq=bq, Wk=Wk, bk=bk, Wv=Wv, bv=bv,
                          Wo=Wo, bo=bo))
    return y


# revision 14
# speedup vs baseline: 1.0064x; 1.0064x over previous
"""Trainium2 Bass kernel for nn_DynamicSparseAttention (B=2,S=1024,E=1024,H=16,K=64).

Sharding: 8 cores = 2 batches x 4 head-groups (4 heads each).
Per core: QKV projections for its 4 heads (f32 for Q/K since top-k selection is
precision-critical, bf16 for V), per-head scores, exact top-64 per query row via
8 rounds of DVE max8 + match_replace, masked softmax with normalization folded
into the ACT exp bias, DMA-transposed weights -> AV matmul, AllGather of head
outputs within each batch group, then full output projection.
"""

import sys

if "/opt/trn_rl_repo" not in sys.path:
    sys.path.insert(0, "/opt/trn_rl_repo")

import numpy as np
import ml_dtypes


def _install_ntff_hook_module():
    """bass_utils(trace=True) imports antenv.axon_hooks, which this image's
    read-only antenv lacks; provide it via sys.modules (ctypes into
    libaxon_pjrt.so, same recipe as the boot script)."""
    import types, contextlib, ctypes

    if "antenv.axon_hooks" in sys.modules:
        return
    mod = types.ModuleType("antenv.axon_hooks")
    state = {"hook": None}

    def _make_hook(so_path="/opt/axon/libaxon_pjrt.so"):
        lib = ctypes.CDLL(so_path)
        if not hasattr(lib, "axon_start_nrt_profile"):
            return None
        lib.axon_start_nrt_profile.argtypes = [
            ctypes.POINTER(ctypes.c_int64), ctypes.c_size_t]
        lib.axon_start_nrt_profile.restype = ctypes.c_int64
        lib.axon_stop_nrt_profile.argtypes = [ctypes.c_char_p]
        lib.axon_stop_nrt_profile.restype = ctypes.c_int64

        @contextlib.contextmanager
        def _hook(output_dir, device_ids):
            import jax
            jax.devices()
            if device_ids:
                ids = (ctypes.c_int64 * len(device_ids))(*device_ids)
                rc = lib.axon_start_nrt_profile(ids, len(device_ids))
            else:
                rc = lib.axon_start_nrt_profile(None, 0)
            if rc != 0:
                raise RuntimeError(f"axon_start_nrt_profile rc={rc}")
            try:
                yield
            finally:
                n = lib.axon_stop_nrt_profile(str(output_dir).encode())
                print(f"profile: {n} file(s) -> {output_dir}", file=sys.stderr)

        return _hook

    def get_axon_ntff_profile_hook():
        if state["hook"] is None:
            try:
                state["hook"] = _make_hook()
            except OSError:
                state["hook"] = None
        return state["hook"]

    def set_axon_ntff_profile_hook(hook):
        state["hook"] = hook

    mod.get_axon_ntff_profile_hook = get_axon_ntff_profile_hook
    mod.set_axon_ntff_profile_hook = set_axon_ntff_profile_hook
    sys.modules["antenv.axon_hooks"] = mod
    try:
        import antenv
        antenv.axon_hooks = mod
    except ImportError:
        pass


_install_ntff_hook_module()

B, S, E = 2, 1024, 1024
H, HD, TOPK = 16, 64, 64
NCORES = 8
HPC = 4          # heads per core
DL = HPC * HD    # 256 local e dims per core
NT = E // 128    # 8 e-tiles
SCALE = 1.0 / 8.0  # 1/sqrt(hd)

BF16 = ml_dtypes.bfloat16

_CACHE = {}


def _build_nc():
    import concourse.bass as bass
    import concourse.bacc as bacc
    import concourse.tile as tile
    from concourse import mybir

    f32 = mybir.dt.float32
    bf16 = mybir.dt.bfloat16
    AF = mybir.ActivationFunctionType
    OP = mybir.AluOpType

    nc = bacc.Bacc("TRN2", target_bir_lowering=False, debug=False,
                   num_devices=NCORES)

    xT_d = nc.dram_tensor("xT", [E, S], f32, kind="ExternalInput")
    xTb_d = nc.dram_tensor("xTb", [E, S], bf16, kind="ExternalInput")
    wqT_d = nc.dram_tensor("wqT", [E, DL], f32, kind="ExternalInput")
    wkT_d = nc.dram_tensor("wkT", [E, DL], f32, kind="ExternalInput")
    wvT_d = nc.dram_tensor("wvT", [E, DL], bf16, kind="ExternalInput")
    woT_d = nc.dram_tensor("woT", [E, E], bf16, kind="ExternalInput")
    bq_d = nc.dram_tensor("bq", [DL, 1], f32, kind="ExternalInput")
    bk_d = nc.dram_tensor("bk", [DL, 1], f32, kind="ExternalInput")
    bv_d = nc.dram_tensor("bv", [E, 1], bf16, kind="ExternalInput")
    bo_d = nc.dram_tensor("bo", [1, E], f32, kind="ExternalInput")
    y_d = nc.dram_tensor("y", [S, E], f32, kind="ExternalOutput")

    outT_int = [nc.dram_tensor(f"outT_int{i}", [128, S], bf16) for i in range(2)]
    ag_out = [nc.dram_tensor(f"ag_out{i}", [512, S], bf16) for i in range(2)]
    groups = [[0, 1, 2, 3], [4, 5, 6, 7]]

    with tile.TileContext(nc) as tc:
        with tc.tile_pool(name="persist", bufs=1) as pp, \
             tc.tile_pool(name="psum", bufs=1, space="PSUM") as psp:
            qt_sb = [pp.tile([128, S], f32, tag=f"qt{p}", name=f"qtsb{p}")
                     for p in range(2)]
            kt_sb = [pp.tile([128, S], f32, tag=f"kt{p}", name=f"ktsb{p}")
                     for p in range(2)]
            v_sb = pp.tile([128, NT * DL], bf16, tag="v")
            outT_sb = [pp.tile([64, S], bf16, tag=f"ot{h}", name=f"outTsb{h}")
                       for h in range(HPC)]
            ones_sb = pp.tile([1, 128], bf16, tag="ones")
            onec_sb = pp.tile([128, 1], f32, tag="onec")
            nc.vector.memset(onec_sb[:], 1.0)
            wo_sb = pp.tile([128, NT * E], bf16, tag="wo")
            bv_sb = pp.tile([128, NT], bf16, tag="bv")
            bo_sb = pp.tile([1, E], f32, tag="bo")
            c_sb = pp.tile([1, E], bf16, tag="c")
            nc.vector.memset(ones_sb[:], 1.0)

            wo_t = woT_d.ap().rearrange("(t p) e -> t p e", p=128)
            bv_t = bv_d.ap().rearrange("(t p) o -> t p o", p=128)
            for t in range(NT):
                nc.sync.dma_start(wo_sb[:, t * E:(t + 1) * E], wo_t[t])
                nc.sync.dma_start(bv_sb[:, t:t + 1], bv_t[t])
            nc.sync.dma_start(bo_sb[:], bo_d.ap())

            # ------------- phase A: projections -------------
            with tc.tile_pool(name="phaseA", bufs=1) as pa:
                xT_sb = pa.tile([128, NT * S], f32, tag="xT")
                xTb_sb = pa.tile([128, NT * S], bf16, tag="xTb")
                wq_sb = pa.tile([128, NT * DL], f32, tag="wq")
                wk_sb = pa.tile([128, NT * DL], f32, tag="wk")
                wv_sb = pa.tile([128, NT * DL], bf16, tag="wv")
                bq_sb = pa.tile([128, 2], f32, tag="bq")
                bk_sb = pa.tile([128, 2], f32, tag="bk")

                xT_t = xT_d.ap().rearrange("(t p) s -> t p s", p=128)
                xTb_t = xTb_d.ap().rearrange("(t p) s -> t p s", p=128)
                wq_t = wqT_d.ap().rearrange("(t p) d -> t p d", p=128)
                wk_t = wkT_d.ap().rearrange("(t p) d -> t p d", p=128)
                wv_t = wvT_d.ap().rearrange("(t p) d -> t p d", p=128)
                for t in range(NT):
                    nc.sync.dma_start(xT_sb[:, t * S:(t + 1) * S], xT_t[t])
                    nc.sync.dma_start(xTb_sb[:, t * S:(t + 1) * S], xTb_t[t])
                    nc.sync.dma_start(wq_sb[:, t * DL:(t + 1) * DL], wq_t[t])
                    nc.sync.dma_start(wk_sb[:, t * DL:(t + 1) * DL], wk_t[t])
                    nc.sync.dma_start(wv_sb[:, t * DL:(t + 1) * DL], wv_t[t])
                bq_t = bq_d.ap().rearrange("(h p) o -> h p o", p=128)
                bk_t = bk_d.ap().rearrange("(h p) o -> h p o", p=128)
                for p in range(2):
                    nc.sync.dma_start(bq_sb[:, p:p + 1], bq_t[p])
                    nc.sync.dma_start(bk_sb[:, p:p + 1], bk_t[p])

                def qk_proj(p):
                    for (w_sb, b_sb, dst) in ((wk_sb, bk_sb, kt_sb),
                                              (wq_sb, bq_sb, qt_sb)):
                        for nb in range(2):
                            ps = psp.tile([128, 512], f32, tag="small",
                                          bufs=2, name=f"pj{p}{nb}")
                            for t in range(NT):
                                nc.tensor.matmul(
                                    ps[:],
                                    w_sb[:, t * DL + p * 128: t * DL + (p + 1) * 128],
                                    xT_sb[:, t * S + nb * 512: t * S + nb * 512 + 512],
                                    start=(t == 0), stop=(t == NT - 1))
                            nc.scalar.activation(
                                dst[p][:, nb * 512:(nb + 1) * 512], ps[:],
                                AF.Identity, bias=b_sb[:, p:p + 1])

                def v_proj():
                    for kt in range(NT):
                        ps = psp.tile([128, DL], f32, tag="small", bufs=2,
                                      name=f"vp{kt}")
                        for t in range(NT):
                            nc.tensor.matmul(
                                ps[:],
                                xTb_sb[:, t * S + kt * 128: t * S + (kt + 1) * 128],
                                wv_sb[:, t * DL:(t + 1) * DL],
                                start=(t == 0), stop=(t == NT - 1))
                        nc.scalar.activation(v_sb[:, kt * DL:(kt + 1) * DL],
                                             ps[:], AF.Copy)

                qk_proj(0)
                v_proj()

                # ------------- phase B: attention per head -------------
                with tc.tile_pool(name="sco", bufs=3) as sco_p, \
                     tc.tile_pool(name="zap", bufs=3) as zap_p, \
                     tc.tile_pool(name="msk", bufs=3) as msk_p, \
                     tc.tile_pool(name="prob", bufs=3) as prob_p, \
                     tc.tile_pool(name="pmw", bufs=3) as pm_p, \
                     tc.tile_pool(name="small", bufs=8) as sm_p, \
                     tc.tile_pool(name="pmT", bufs=2) as pmT_p:
                    for h in range(HPC):
                        pair, sub = h // 2, h % 2
                        r0 = sub * 64
                        if h == 1:
                            qk_proj(1)   # overlap pair-1 proj with head work
                        pmT = pmT_p.tile([128, NT * S], bf16, tag="pmT",
                                         name=f"pmT{h}")
                        for qp in range(NT // 2):
                            qts = (2 * qp, 2 * qp + 1)
                            sps_l, s_l, z_l, t64_l = [], [], [], []
                            for qt in qts:
                                sps = psp.tile([128, S], f32, tag="big",
                                               bufs=3, name=f"sps{h}{qt}")
                                for nb in range(2):
                                    nc.tensor.matmul(
                                        sps[:, nb * 512:(nb + 1) * 512],
                                        qt_sb[pair][r0:r0 + 64, qt * 128:(qt + 1) * 128],
                                        kt_sb[pair][r0:r0 + 64, nb * 512:(nb + 1) * 512],
                                        start=True, stop=True)
                                sps_l.append(sps)
                                s_l.append(sco_p.tile([128, S], f32, tag="s",
                                                      name=f"s{h}{qt}"))
                                z_l.append(zap_p.tile([128, S], f32, tag="z",
                                                      name=f"z{h}{qt}"))
                                t64_l.append(sm_p.tile([128, 64], f32,
                                                       tag=f"t64_{qt % 2}",
                                                       name=f"t64_{h}{qt}"))
                            # interleaved exact top-64 extraction for the two
                            # q-tiles: every needle-load (MATCH_VALUE_LOAD)
                            # hides behind the other tile's big scan, so the
                            # DVE pipe-drain bubbles vanish
                            nc.vector.max(t64_l[0][:, 0:8], sps_l[0][:])
                            nc.vector.max(t64_l[1][:, 0:8], sps_l[1][:])
                            nc.vector.match_replace(z_l[0][:], t64_l[0][:, 0:8],
                                                    sps_l[0][:], -1e30)
                            nc.vector.match_replace(z_l[1][:], t64_l[1][:, 0:8],
                                                    sps_l[1][:], -1e30)
                            nc.scalar.activation(s_l[0][:], sps_l[0][:], AF.Copy)
                            nc.scalar.activation(s_l[1][:], sps_l[1][:], AF.Copy)
                            for r in range(1, 8):
                                nc.vector.max(t64_l[0][:, 8 * r:8 * r + 8],
                                              z_l[0][:])
                                nc.vector.max(t64_l[1][:, 8 * r:8 * r + 8],
                                              z_l[1][:])
                                if r < 7:
                                    nc.vector.match_replace(
                                        z_l[0][:], t64_l[0][:, 8 * r:8 * r + 8],
                                        z_l[0][:], -1e30)
                                    nc.vector.match_replace(
                                        z_l[1][:], t64_l[1][:, 8 * r:8 * r + 8],
                                        z_l[1][:], -1e30)
                            for i, qt in enumerate(qts):
                                t64, s_sb = t64_l[i], s_l[i]
                                theta = t64[:, 63:64]
                                p_sb = prob_p.tile([128, S], bf16, tag="p",
                                                   name=f"p{h}{qt}")
                                nc.scalar.activation(p_sb[:], s_sb[:], AF.Exp,
                                                     scale=SCALE)
                                e64 = sm_p.tile([128, 64], f32, tag="e64",
                                                name=f"e64_{h}{qt}")
                                den = sm_p.tile([128, 1], f32, tag="den",
                                                name=f"den{h}{qt}")
                                nc.scalar.activation(e64[:], t64[:], AF.Exp,
                                                     scale=SCALE,
                                                     accum_out=den[:])
                                rden = sm_p.tile([128, 1], f32, tag="rden",
                                                 name=f"rden{h}{qt}")
                                nc.vector.reciprocal(rden[:], den[:])
                                m_sb = msk_p.tile([128, S], bf16, tag="m",
                                                  name=f"m{h}{qt}")
                                nc.vector.tensor_scalar(m_sb[:], s_sb[:],
                                                        theta, None,
                                                        op0=OP.is_ge)
                                pm_sb = pm_p.tile([128, S], bf16, tag="pm",
                                                  name=f"pm{h}{qt}")
                                nc.vector.scalar_tensor_tensor(
                                    pm_sb[:], p_sb[:], rden[:], m_sb[:],
                                    op0=OP.mult, op1=OP.mult)
                                for kt in range(NT):
                                    nc.sync.dma_start(
                                        pmT[:, kt * S + qt * 128: kt * S + (qt + 1) * 128],
                                        pm_sb[:, kt * 128:(kt + 1) * 128],
                                        transpose=True)
                        avps = psp.tile([64, S], f32, tag="big", bufs=3,
                                        name=f"avps{h}")
                        for nb in range(2):
                            for kt in range(NT):
                                nc.tensor.matmul(
                                    avps[:, nb * 512:(nb + 1) * 512],
                                    v_sb[:, kt * DL + h * 64: kt * DL + (h + 1) * 64],
                                    pmT[:, kt * S + nb * 512: kt * S + nb * 512 + 512],
                                    start=(kt == 0), stop=(kt == NT - 1))
                        nc.scalar.activation(outT_sb[h][:], avps[:], AF.Copy)
                        nc.sync.dma_start(
                            outT_int[h // 2].ap()[(h % 2) * 64:(h % 2) * 64 + 64, :],
                            outT_sb[h][:])
                        if h % 2 == 1:
                            # AllGather this half; the first one overlaps the
                            # remaining heads' compute
                            nc.gpsimd.collective_compute(
                                "AllGather", mybir.AluOpType.bypass,
                                ins=[outT_int[h // 2].ap()],
                                outs=[ag_out[h // 2].ap()],
                                replica_groups=groups)

            # ------------- phase D: output projection -------------
            with tc.tile_pool(name="phaseD", bufs=1) as pd, \
                 tc.tile_pool(name="ysb", bufs=2) as yp:
                ot_sb = pd.tile([128, NT * S], bf16, tag="ot")
                for t in [0, 2, 4, 6, 1, 3, 5, 7]:
                    r, half = t // 2, t % 2
                    nc.sync.dma_start(
                        ot_sb[:, t * S:(t + 1) * S],
                        ag_out[half].ap()[r * 128:(r + 1) * 128, :])

                # c = bv @ Wo.T + bo   (constant row, [1, E])
                cps = psp.tile([1, E], f32, tag="big", bufs=3, name="cps")
                for nb in range(2):
                    for t in range(NT):
                        nc.tensor.matmul(
                            cps[:, nb * 512:(nb + 1) * 512],
                            bv_sb[:, t:t + 1],
                            wo_sb[:, t * E + nb * 512: t * E + nb * 512 + 512],
                            start=(t == 0), stop=(t == NT - 1))
                nc.vector.tensor_tensor(c_sb[:], cps[:], bo_sb[:], op=OP.add)

                for st in range(NT):
                    yps = psp.tile([128, E], f32, tag="big", bufs=3,
                                   name=f"yps{st}")
                    for nb in range(2):
                        for i, t in enumerate([0, 2, 4, 6, 1, 3, 5, 7]):
                            nc.tensor.matmul(
                                yps[:, nb * 512:(nb + 1) * 512],
                                ot_sb[:, t * S + st * 128: t * S + (st + 1) * 128],
                                wo_sb[:, t * E + nb * 512: t * E + nb * 512 + 512],
                                start=(i == 0), stop=False)
                        nc.tensor.matmul(
                            yps[:, nb * 512:(nb + 1) * 512],
                            ones_sb[:],
                            c_sb[:, nb * 512:(nb + 1) * 512],
                            start=False, stop=True)
                    y_sb = yp.tile([128, E], f32, tag="y", name=f"y{st}")
                    nc.scalar.activation(y_sb[:], yps[:], AF.Copy)
                    nc.sync.dma_start(y_d.ap()[st * 128:(st + 1) * 128, :],
                                      y_sb[:])

    nc.compile()
    return nc


def _get_nc():
    if "nc" not in _CACHE:
        _CACHE["nc"] = _build_nc()
    return _CACHE["nc"]


def _in_maps(x, Wq, bq, Wk, bk, Wv, bv, Wo, bo):
    x = np.asarray(x, np.float32)
    Wq = np.asarray(Wq, np.float32)
    Wk = np.asarray(Wk, np.float32)
    Wv = np.asarray(Wv, np.float32)
    Wo = np.asarray(Wo, np.float32)
    bq = np.asarray(bq, np.float32)
    bk = np.asarray(bk, np.float32)
    bv = np.asarray(bv, np.float32)
    bo = np.asarray(bo, np.float32)

    woT = np.ascontiguousarray(Wo.T).astype(BF16)
    bo_r = bo.reshape(1, E)
    bv_r = bv.reshape(E, 1).astype(BF16)
    maps = []
    for c in range(NCORES):
        b = c // 4
        dlo = (c % 4) * DL
        xT = np.ascontiguousarray(x[b].T)
        maps.append({
            "xT": xT,
            "xTb": xT.astype(BF16),
            "wqT": np.ascontiguousarray(Wq[dlo:dlo + DL, :].T),
            "wkT": np.ascontiguousarray(Wk[dlo:dlo + DL, :].T),
            "wvT": np.ascontiguousarray(Wv[dlo:dlo + DL, :].T).astype(BF16),
            "woT": woT,
            "bq": np.ascontiguousarray(bq[dlo:dlo + DL].reshape(DL, 1)),
            "bk": np.ascontiguousarray(bk[dlo:dlo + DL].reshape(DL, 1)),
            "bv": bv_r,
            "bo": bo_r,
        })
    return maps


def run_on_hw(inputs, trace=False):
    """Run the bass kernel; returns (output, BassKernelResults)."""
    from concourse.bass_utils import run_bass_kernel_spmd

    nc = _get_nc()
    maps = _in_maps(**inputs)
    res = run_bass_kernel_spmd(nc, maps, core_ids=list(range(NCORES)),
                               trace=trace)
    y = np.stack([np.asarray(res.results[0]["y"]),
                  np.asarray(res.results[4]["y"])]).astype(np.float32)
    return y, res


def kernel(x, Wq, bq, Wk, bk, Wv, bv, Wo, bo):
    y, _ = run_on_hw(dict(x=x, Wq=Wq, bq=bq, Wk=Wk, bk=bk, Wv=Wv, bv=bv,
                          Wo=Wo, bo=bo))
    return y
